# revision 20
# baseline (speedup 1.0000x reference)
"""Tensor-parallel causal attention block (dense transformer) on 8 TRN2 NeuronCores.

Strategy (tensor parallel over heads, 4 heads / core):
  - Host pre-transposes x -> xT (din on rows) and the per-core weight shards
    (wq/wk/wv column shards, wo row shard), so every on-device matmul has its
    contraction dim on SBUF partitions with no on-device transposes of x/w.
  - Per core: qT/kT projections emitted in head-transposed layout (d x tok),
    rotary applied via a pair-swap permutation matmul + cos/sin elementwise;
    v projected then PE-transposed to natural (tok x d) layout.
  - Attention per (batch, local head) with scores computed TRANSPOSED
    (kv on partitions, q on free axis): softmax needs no cross-partition max
    (scores are bounded, exp(-1e9) underflows to 0 for the causal mask), the
    denominator comes from a ones-column matmul, and probs feed the PV matmul
    directly without any transposes.
  - Local head outputs are normalized, cast to bf16, AllGather'ed across the
    8 cores (4 chunks, overlapped with the second batch's attention + wo
    matmuls), then each core computes its 512-wide column shard of out @ wo.T
    in bf16.
  - Projections/attention matmuls run as float32r (fp22 reduced precision,
    full PE rate at free-dim >= 256) with fp32 PSUM accumulation.

Outputs returned per core and re-assembled (transpose/concat) on the host.
"""

import math
import os
import sys

import numpy as np

for _p in ("/opt/trn_rl_repo", "/root/.axon_site/_ro/trn_rl_repo"):
    if os.path.isdir(_p) and _p not in sys.path:
        sys.path.insert(0, _p)

N_CORES = 8
FULL = dict(D=4096, S=2048, B=2, H=32)

_PROG_CACHE = {}


def build_program(D, S, B, H):
    """Build + compile the SPMD Bass program. Returns (nc, meta)."""
    import concourse.bass as bass
    import concourse.tile as tile
    from concourse import bacc, mybir

    f32 = mybir.dt.float32
    f32r = mybir.dt.float32r
    bf16 = mybir.dt.bfloat16
    Exp = mybir.ActivationFunctionType.Exp

    NC = N_CORES
    TOK = B * S
    HL = H // NC          # local heads
    DL = HL * 128         # local width of q/k/v/out shards
    assert H * 128 == D
    ND = D // 128         # contraction tiles
    NM = DL // 128        # local dout tiles
    TOKC = 512            # token chunk (matmul free dim)
    NTC = TOK // TOKC
    QC = 512              # q chunk in attention
    NQC = S // QC
    NKI = S // 128        # kv tiles per head
    XG = min(8, ND)       # din tiles per x-slab DMA
    NG = ND // XG
    AGC = min(512, TOK)   # allgather chunk width (tokens)
    NAG = TOK // AGC
    NB = QC // 128        # causal band masks
    WCH = 8               # weight-slab dt tiles per DMA chunk
    scale = 1.0 / math.sqrt(128.0)

    nc = bacc.Bacc("TRN2", target_bir_lowering=False, debug=False, num_devices=NC)

    def din(name, shape, dt=f32):
        return nc.dram_tensor(name, list(shape), dt, kind="ExternalInput").ap()

    xT = din("xT", (D, TOK))
    wT = {p: din(f"w{p}T", (D, DL)) for p in ("q", "k", "v")}
    woT = din("woT", (D, DL), bf16)
    cmat = din("cmat", (128, S))
    smat = din("smat", (128, S))
    consts = din("consts", (128, 512))
    onesb = din("onesb", (1, 128), bf16)
    maskb = din("maskb", (128, NB * 512))

    out_t = nc.dram_tensor("out_t", [DL, TOK], f32, kind="ExternalOutput").ap()
    k_t = nc.dram_tensor("k_t", [DL, TOK], f32, kind="ExternalOutput").ap()
    v_n = nc.dram_tensor("v_n", [TOK, DL], f32, kind="ExternalOutput").ap()
    q_t = nc.dram_tensor("q_t", [DL, TOK], f32).ap()
    ag_in = [nc.dram_tensor(f"ag_in{j}", [DL, AGC], bf16).ap() for j in range(NAG)]
    ag_out = [
        nc.dram_tensor(f"ag_out{j}", [D, AGC], bf16, addr_space="Shared").ap()
        for j in range(NAG)
    ]

    xT_r = xT.bitcast(f32r).rearrange("(dt p) t -> p dt t", p=128)

    def load_w_slab(pool, wap, dtype, tag):
        """Weight slab [128, ND*DL], chunked so early matmuls unblock fast."""
        t = pool.tile([128, ND * DL], dtype, tag=tag, name=f"{tag}_sb")
        tr = t[:].rearrange("p (dt j) -> p dt j", dt=ND)
        wr = wap.bitcast(dtype) if dtype == f32r else wap
        wr = wr.rearrange("(dt p) j -> p dt j", p=128)
        for c0 in range(0, ND, WCH):
            c1 = min(c0 + WCH, ND)
            nc.sync.dma_start(tr[:, c0:c1], wr[:, c0:c1])
        return t

    with tile.TileContext(nc) as tc:
        with tc.tile_pool(name="const", bufs=1) as cp:
            ccr = cp.tile([128, 129], f32r)
            nc.sync.dma_start(ccr[:], consts.bitcast(f32r)[:, 0:129])
            pt_r = ccr[:, 0:128]
            ones_col_r = ccr[:, 128:129]
            cc = cp.tile([128, 257], f32)
            nc.sync.dma_start(cc[:], consts[:, 129:386])
            ident_sb = cc[:, 0:128]
            ones_row = cc[0:1, 128:256]
            ones_col_f = cc[:, 256:257]
            onesb_sb = cp.tile([1, 128], bf16)
            nc.sync.dma_start(onesb_sb[:], onesb)
            mb_sb = cp.tile([128, NB * 512], f32)
            nc.sync.dma_start(mb_sb[:], maskb)

            # ---------- P1: Q,K projections + rotary (transposed layout) ----
            with tc.tile_pool(name="p1c", bufs=1) as c1p, \
                 tc.tile_pool(name="p1w", bufs=1) as wp, \
                 tc.tile_pool(name="p1x", bufs=2) as xp, \
                 tc.tile_pool(name="p1o", bufs=3) as op_, \
                 tc.tile_pool(name="p1ps", bufs=2 * NM, space="PSUM") as pp:
                cmat_sb = c1p.tile([128, S], f32)
                nc.sync.dma_start(cmat_sb[:], cmat)
                smat_sb = c1p.tile([128, S], f32)
                nc.sync.dma_start(smat_sb[:], smat)
                # first x slab before the weight slabs: matmuls need both,
                # and this orders the DMA queue so dt=0 work unblocks early
                xg0 = xp.tile([128, XG * TOKC], f32r, tag="xg", name="xg_0_0")
                nc.sync.dma_start(
                    xg0[:].rearrange("p (i t) -> p i t", i=XG),
                    xT_r[:, 0:XG, 0:TOKC],
                )
                w_sb = {}
                w_tr = {}
                for p in ("q", "k"):
                    t = wp.tile([128, ND * DL], f32r, tag=f"w{p}", name=f"w{p}_sb")
                    w_sb[p] = t
                    w_tr[p] = (t[:].rearrange("p (dt j) -> p dt j", dt=ND),
                               wT[p].bitcast(f32r).rearrange("(dt p) j -> p dt j", p=128))
                for c0 in range(0, ND, WCH):
                    c1 = min(c0 + WCH, ND)
                    for p in ("q", "k"):
                        tr, wr = w_tr[p]
                        nc.sync.dma_start(tr[:, c0:c1], wr[:, c0:c1])
                projs = (("q", q_t), ("k", k_t))
                for tokc in range(NTC):
                    scol = (tokc * TOKC) % S
                    xg = []
                    for g in range(NG):
                        if tokc == 0 and g == 0:
                            xg.append(xg0)
                            continue
                        t = xp.tile([128, XG * TOKC], f32r, tag="xg", name=f"xg_{tokc}_{g}")
                        nc.sync.dma_start(
                            t[:].rearrange("p (i t) -> p i t", i=XG),
                            xT_r[:, g * XG:(g + 1) * XG, tokc * TOKC:(tokc + 1) * TOKC],
                        )
                        xg.append(t)
                    psums = {}
                    for pname, _ in projs:
                        for m in range(NM):
                            psums[(pname, m)] = pp.tile([128, TOKC], f32, tag="pr", name=f"ps_{tokc}_{pname}{m}")
                    for dt in range(ND):
                        g, i = divmod(dt, XG)
                        rhs = xg[g][:, i * TOKC:(i + 1) * TOKC]
                        for pname, _ in projs:
                            for m in range(NM):
                                nc.tensor.matmul(
                                    psums[(pname, m)][:],
                                    w_sb[pname][:, dt * DL + m * 128: dt * DL + (m + 1) * 128],
                                    rhs,
                                    start=(dt == 0), stop=(dt == ND - 1),
                                )
                    for pname, dst in projs:
                        for m in range(NM):
                            ps = psums[(pname, m)]
                            raw = op_.tile([128, TOKC], f32r, tag="raw")
                            nc.scalar.copy(raw[:], ps[:])
                            psw = pp.tile([128, TOKC], f32, tag="pr")
                            nc.tensor.matmul(psw[:], pt_r, raw[:], start=True, stop=True)
                            t2 = op_.tile([128, TOKC], f32, tag="t2")
                            nc.vector.tensor_mul(t2[:], psw[:], smat_sb[:, scol:scol + TOKC])
                            t1 = op_.tile([128, TOKC], f32, tag="t1")
                            nc.vector.tensor_mul(t1[:], raw[:].bitcast(f32), cmat_sb[:, scol:scol + TOKC])
                            nc.vector.tensor_add(t1[:], t1[:], t2[:])
                            nc.sync.dma_start(
                                dst[m * 128:(m + 1) * 128, tokc * TOKC:(tokc + 1) * TOKC],
                                t1[:],
                            )

            # ---------- P2+P3+P4: V projection, attention, AllGather, wo ----
            # One nested scope tree so the v-projection matmuls fill batch-0
            # attention slack and the wo matmuls fill batch-1 slack.
            with tc.tile_pool(name="p3qkv", bufs=2) as qp, \
                 tc.tile_pool(name="p3pr", bufs=min(14, NKI + 2)) as prp, \
                 tc.tile_pool(name="p3o", bufs=3) as op_, \
                 tc.tile_pool(name="p3a", bufs=2) as accp, \
                 tc.tile_pool(name="p3ps", bufs=2, space="PSUM") as psc, \
                 tc.tile_pool(name="p3po", bufs=1, space="PSUM") as pso, \
                 tc.tile_pool(name="p3pl", bufs=1, space="PSUM") as psl:
                v_n_r = v_n.bitcast(f32r).rearrange("(t2 p) j -> p t2 j", p=128)
                NQH = (NQC + 1) // 2

                def emit_v_chunks(wv_sb, xp, vop, pp, tokcs):
                    for tokc in tokcs:
                        xg = []
                        for g in range(NG):
                            t = xp.tile([128, XG * TOKC], f32r, tag="xg", name=f"xg2_{tokc}_{g}")
                            nc.sync.dma_start(
                                t[:].rearrange("p (i t) -> p i t", i=XG),
                                xT_r[:, g * XG:(g + 1) * XG, tokc * TOKC:(tokc + 1) * TOKC],
                            )
                            xg.append(t)
                        psums = [pp.tile([128, TOKC], f32, tag="pv", name=f"psv_{tokc}_{m}") for m in range(NM)]
                        for dt in range(ND):
                            g, i = divmod(dt, XG)
                            rhs = xg[g][:, i * TOKC:(i + 1) * TOKC]
                            for m in range(NM):
                                nc.tensor.matmul(
                                    psums[m][:],
                                    wv_sb[:, dt * DL + m * 128: dt * DL + (m + 1) * 128],
                                    rhs,
                                    start=(dt == 0), stop=(dt == ND - 1),
                                )
                        vT_sb = []
                        for m in range(NM):
                            t = vop.tile([128, TOKC], f32, tag="vT", name=f"vT_{tokc}_{m}")
                            nc.scalar.copy(t[:], psums[m][:])
                            vT_sb.append(t)
                        for tt in range(TOKC // 128):
                            ptr = pp.tile([128, NM * 128], f32, tag="pv", name=f"ptr_{tokc}_{tt}")
                            for m in range(NM):
                                nc.tensor.transpose(
                                    ptr[:, m * 128:(m + 1) * 128],
                                    vT_sb[m][:, tt * 128:(tt + 1) * 128],
                                    ident_sb,
                                )
                            vn = vop.tile([128, NM * 128], f32, tag="vn")
                            nc.vector.tensor_copy(vn[:], ptr[:])
                            nc.sync.dma_start(
                                v_n[tokc * TOKC + tt * 128: tokc * TOKC + (tt + 1) * 128, :],
                                vn[:],
                            )

                def emit_attn_batch(b):
                    for qch in range(NQH):
                        qcs = [qc for qc in (2 * qch, 2 * qch + 1) if qc < NQC]
                        nki_max = min(4 * (qcs[-1] + 1), NKI)
                        for hl in range(HL):
                            qT_sb = qp.tile([128, len(qcs) * QC], f32r, tag="qT",
                                            name=f"qT_{b}_{qch}_{hl}")
                            nc.sync.dma_start(
                                qT_sb[:],
                                q_t.bitcast(f32r)[hl * 128:(hl + 1) * 128,
                                                  b * S + qcs[0] * QC: b * S + (qcs[-1] + 1) * QC])
                            kT_sb = qp.tile([128, nki_max * 128], f32r, tag="kT",
                                            name=f"kT_{b}_{qch}_{hl}")
                            nc.sync.dma_start(
                                kT_sb[:],
                                k_t.bitcast(f32r)[hl * 128:(hl + 1) * 128,
                                                  b * S: b * S + nki_max * 128])
                            v_sb = qp.tile([128, nki_max * 128], f32r, tag="v",
                                           name=f"v_{b}_{qch}_{hl}")
                            nc.sync.dma_start(
                                v_sb[:].rearrange("p (ki j) -> p ki j", ki=nki_max),
                                v_n_r[:, b * NKI: b * NKI + nki_max, hl * 128:(hl + 1) * 128],
                            )
                            for qc in qcs:
                                nki_q = min(4 * (qc + 1), NKI)
                                qoff = (qc - qcs[0]) * QC
                                probs = []
                                acc = accp.tile([128, QC], f32, tag="acc")
                                for ki in range(nki_q):
                                    p_sc = psc.tile([128, QC], f32, tag="sc")
                                    nc.tensor.matmul(
                                        p_sc[:],
                                        kT_sb[:, ki * 128:(ki + 1) * 128],
                                        qT_sb[:, qoff:qoff + QC],
                                        start=True, stop=True,
                                    )
                                    j = ki - (qc * QC) // 128
                                    if j >= 0:
                                        nc.vector.tensor_add(p_sc[:], p_sc[:], mb_sb[:, j * 512:(j + 1) * 512])
                                    pt_ = prp.tile([128, QC], f32r, tag="probs",
                                                   name=f"probs_{b}_{hl}_{qc}_{ki}")
                                    nc.scalar.activation(pt_[:], p_sc[:], Exp, scale=scale)
                                    if ki == 0:
                                        nc.scalar.copy(acc[:], pt_[:].bitcast(f32))
                                    else:
                                        nc.vector.tensor_add(acc[:], acc[:], pt_[:].bitcast(f32))
                                    probs.append(pt_)
                                p_out = pso.tile([128, QC], f32, tag="po")
                                p_l = psl.tile([1, QC], f32, tag="pl")
                                for ki in range(nki_q):
                                    nc.tensor.matmul(
                                        p_out[:],
                                        v_sb[:, ki * 128:(ki + 1) * 128],
                                        probs[ki][:],
                                        start=(ki == 0), stop=(ki == nki_q - 1),
                                    )
                                nc.tensor.matmul(p_l[:], ones_col_f, acc[:], start=True, stop=True)
                                r_sb = op_.tile([1, QC], bf16, tag="r")
                                with nc.allow_low_precision(reason="1/l broadcast in bf16 is intentional"):
                                    nc.vector.reciprocal(r_sb[:], p_l[:])
                                p_r = psc.tile([128, QC], f32, tag="sc")
                                nc.tensor.matmul(p_r[:], onesb_sb[:], r_sb[:], start=True, stop=True)
                                rbc = op_.tile([128, QC], f32, tag="rbc")
                                nc.scalar.copy(rbc[:], p_r[:])
                                attn = op_.tile([128, QC], bf16, tag="attn")
                                nc.vector.tensor_mul(attn[:], p_out[:], rbc[:])
                                tok0 = b * S + qc * QC
                                j = tok0 // AGC
                                col = tok0 % AGC
                                nc.sync.dma_start(
                                    ag_in[j][hl * 128:(hl + 1) * 128, col:col + QC], attn[:]
                                )
                                if hl == HL - 1:
                                    for jj in range(NAG):
                                        last_b = min(B - 1, (AGC * (jj + 1) - 1) // S)
                                        last_qc = min(NQC - 1, (min(AGC * (jj + 1), (last_b + 1) * S) - 1 - last_b * S) // QC)
                                        if last_b == b and last_qc == qc:
                                            nc.gpsimd.collective_compute(
                                                "AllGather",
                                                mybir.AluOpType.bypass,
                                                replica_groups=[list(range(NC))],
                                                ins=[ag_in[jj].opt()],
                                                outs=[ag_out[jj].opt()],
                                            )

                with tc.tile_pool(name="p2w", bufs=1) as wpv, \
                     tc.tile_pool(name="p2x", bufs=2) as xpv, \
                     tc.tile_pool(name="p2o", bufs=NM) as vop, \
                     tc.tile_pool(name="p2ps", bufs=max(NM, TOKC // 128), space="PSUM") as ppv:
                    wv_sb = load_w_slab(wpv, wT["v"], f32r, "wv")
                    emit_v_chunks(wv_sb, xpv, vop, ppv, range(0, NTC // B))
                    emit_attn_batch(0)
                    emit_v_chunks(wv_sb, xpv, vop, ppv, range(NTC // B, NTC))

                # ---------- P4 (pools take over the v-projection's space) ----
                with tc.tile_pool(name="p4w", bufs=1) as wp4, \
                     tc.tile_pool(name="p4x", bufs=5) as xp4, \
                     tc.tile_pool(name="p4o", bufs=2) as op4, \
                     tc.tile_pool(name="p4ps", bufs=min(NM, 4), space="PSUM") as pp4:
                    wo_sb = load_w_slab(wp4, woT, bf16, "wo")
                    for bb in range(1, B):
                        emit_attn_batch(bb)
                    ag_out_r = [a.rearrange("(dt p) t -> p dt t", p=128) for a in ag_out]
                    for tokc in range(NTC):
                        j = (tokc * TOKC) // AGC
                        col = (tokc * TOKC) % AGC
                        xg = []
                        for g in range(NG):
                            t = xp4.tile([128, XG * TOKC], bf16, tag="ag", name=f"agx_{tokc}_{g}")
                            nc.sync.dma_start(
                                t[:].rearrange("p (i t) -> p i t", i=XG),
                                ag_out_r[j][:, g * XG:(g + 1) * XG, col:col + TOKC],
                            )
                            xg.append(t)
                        psums = {m: pp4.tile([128, TOKC], f32, tag="pw", name=f"psw_{tokc}_{m}") for m in range(NM)}
                        for dt in range(ND):
                            g, i = divmod(dt, XG)
                            rhs = xg[g][:, i * TOKC:(i + 1) * TOKC]
                            for m in range(NM):
                                nc.tensor.matmul(
                                    psums[m][:],
                                    wo_sb[:, dt * DL + m * 128: dt * DL + (m + 1) * 128],
                                    rhs,
                                    start=(dt == 0), stop=(dt == ND - 1),
                                )
                        for m in range(NM):
                            ow = op4.tile([128, TOKC], f32, tag="ow")
                            nc.scalar.copy(ow[:], psums[m][:])
                            nc.sync.dma_start(
                                out_t[m * 128:(m + 1) * 128, tokc * TOKC:(tokc + 1) * TOKC],
                                ow[:],
                            )

    nc.compile()
    meta = dict(D=D, S=S, B=B, H=H, HL=HL, DL=DL, NB=NB)
    return nc, meta


def make_inputs(x, freqs_cis, wq, wk, wv, wo, D, S, B, H):
    """Host-side sharding: build the per-core in_maps."""
    import ml_dtypes

    NC = N_CORES
    TOK = B * S
    HL = H // NC
    DL = HL * 128
    NB = 4

    xT = np.ascontiguousarray(x.reshape(TOK, D).T)

    fc = np.asarray(freqs_cis, np.float32)           # (S, 64, 2)
    cmat = np.repeat(np.ascontiguousarray(fc[:, :, 0].T), 2, axis=0).astype(np.float32)
    smat = np.repeat(np.ascontiguousarray(fc[:, :, 1].T), 2, axis=0).astype(np.float32)

    P = np.zeros((128, 128), np.float32)
    idx = np.arange(64)
    P[2 * idx, 2 * idx + 1] = -1.0
    P[2 * idx + 1, 2 * idx] = 1.0
    consts = np.zeros((128, 512), np.float32)
    consts[:, 0:128] = P.T
    consts[:, 128] = 1.0                     # ones_col (f32r view)
    consts[:, 129:257] = np.eye(128, dtype=np.float32)
    consts[0, 257:385] = 1.0                 # ones_row
    consts[:, 385] = 1.0                     # ones_col (f32)

    maskb = np.zeros((128, NB * 512), np.float32)
    p = np.arange(128)[:, None]
    f = np.arange(512)[None, :]
    for j in range(NB):
        maskb[:, j * 512:(j + 1) * 512] = np.where(128 * j + p <= f, 0.0, -1e9)

    in_maps = []
    for c in range(NC):
        sl = slice(c * DL, (c + 1) * DL)
        in_maps.append({
            "xT": xT,
            "wqT": np.ascontiguousarray(wq[sl, :].T),
            "wkT": np.ascontiguousarray(wk[sl, :].T),
            "wvT": np.ascontiguousarray(wv[sl, :].T),
            "woT": np.ascontiguousarray(wo[sl, :].T).astype(ml_dtypes.bfloat16),
            "cmat": cmat,
            "smat": smat,
            "consts": consts,
            "onesb": np.ones((1, 128), ml_dtypes.bfloat16),
            "maskb": maskb,
        })
    return in_maps


def assemble_outputs(results, D, S, B, H):
    """Gather per-core outputs into (out, new_hidden)."""
    NC = N_CORES
    TOK = B * S
    HL = H // NC
    DL = HL * 128

    out = np.empty((TOK, D), np.float32)
    k_full = np.empty((B, S, H, 128), np.float32)
    v_full = np.empty((B, S, H, 128), np.float32)
    for c in range(NC):
        r = results[c]
        out[:, c * DL:(c + 1) * DL] = np.asarray(r["out_t"], np.float32).T
        kt = np.asarray(r["k_t"], np.float32).reshape(HL, 128, B, S).transpose(2, 3, 0, 1)
        k_full[:, :, c * HL:(c + 1) * HL, :] = kt
        v_full[:, :, c * HL:(c + 1) * HL, :] = np.asarray(r["v_n"], np.float32).reshape(B, S, HL, 128)
    out = out.reshape(B, S, D)
    new_hidden = np.stack([k_full, v_full], axis=0)
    return out, new_hidden


def _reference_fallback(x, freqs_cis, mask, hidden_state, wq, wk, wv, wo, start_pos):
    """Pure-numpy mirror of the reference for inputs the fast path can't take."""
    bsz, seqlen, dim = x.shape
    h, hd = dim // 128, 128
    xt = x.reshape(bsz * seqlen, dim).astype(np.float32)
    q = (xt @ wq.T).reshape(bsz, seqlen, h, hd)
    k = (xt @ wk.T).reshape(bsz, seqlen, h, hd)
    v = (xt @ wv.T).reshape(bsz, seqlen, h, hd)
    fc = np.asarray(freqs_cis, np.float32)

    def rot(z):
        zz = z.reshape(bsz, seqlen, h, hd // 2, 2)
        cos = fc[:, :, 0][None, :, None, :]
        sin = fc[:, :, 1][None, :, None, :]
        r = zz[..., 0] * cos - zz[..., 1] * sin
        im = zz[..., 0] * sin + zz[..., 1] * cos
        return np.stack([r, im], -1).reshape(bsz, seqlen, h, hd).astype(np.float32)

    qr, kr = rot(q), rot(k)
    new_k = np.array(hidden_state[0])
    new_v = np.array(hidden_state[1])
    new_k[:bsz, start_pos:start_pos + seqlen] = kr
    new_v[:bsz, start_pos:start_pos + seqlen] = v
    keys = new_k[:bsz, :start_pos + seqlen]
    values = new_v[:bsz, :start_pos + seqlen]
    out = np.zeros((bsz, seqlen, h, hd), np.float32)
    m2 = np.asarray(mask, np.float32)[0, 0]
    for b in range(bsz):
        for hh in range(h):
            sc = (qr[b, :, hh, :] @ keys[b, :, hh, :].T) / math.sqrt(hd) + m2
            sc = sc - sc.max(-1, keepdims=True)
            p = np.exp(sc)
            p = p / p.sum(-1, keepdims=True)
            out[b, :, hh, :] = p @ values[b, :, hh, :]
    return (out.reshape(bsz, seqlen, h * hd) @ wo.T,
            np.stack([new_k, new_v], 0))


def _is_causal_mask(mask, S):
    m = np.asarray(mask, np.float32).reshape(S, S)
    expect = np.triu(np.full((S, S), -1e9, np.float32), k=1)
    return m.shape == expect.shape and np.array_equal(m, expect)


def run_on_device(in_maps, prog_key=None, trace=False):
    from concourse.bass_utils import run_bass_kernel_spmd

    key = prog_key or tuple(sorted(FULL.items()))
    if key not in _PROG_CACHE:
        _PROG_CACHE[key] = build_program(**dict(key))
    nc, meta = _PROG_CACHE[key]
    res = run_bass_kernel_spmd(
        nc, in_maps, core_ids=list(range(N_CORES)), trace=trace
    )
    return res, meta


def kernel(x, freqs_cis, mask, hidden_state, wq, wk, wv, wo, start_pos):
    x = np.asarray(x, np.float32)
    freqs_cis = np.asarray(freqs_cis, np.float32)
    wq = np.asarray(wq, np.float32)
    wk = np.asarray(wk, np.float32)
    wv = np.asarray(wv, np.float32)
    wo = np.asarray(wo, np.float32)
    D, S, B, H = FULL["D"], FULL["S"], FULL["B"], FULL["H"]

    sp = int(start_pos)
    if (sp != 0 or x.shape != (B, S, D) or not _is_causal_mask(mask, S)):
        return _reference_fallback(
            x, freqs_cis, np.asarray(mask), np.asarray(hidden_state),
            wq, wk, wv, wo, sp)

    in_maps = make_inputs(x, freqs_cis, wq, wk, wv, wo, D, S, B, H)
    res, _meta = run_on_device(in_maps)
    out, new_hidden = assemble_outputs(res.results, D, S, B, H)
    return out, new_hidden


# revision 21
# speedup vs baseline: 1.0433x; 1.0433x over previous
"""Tensor-parallel causal attention block (dense transformer) on 8 TRN2 NeuronCores.

Strategy (tensor parallel over heads, 4 heads / core):
  - Host pre-transposes x -> xT (din on rows) and the per-core weight shards
    (wq/wk/wv column shards, wo row shard), so every on-device matmul has its
    contraction dim on SBUF partitions with no on-device transposes of x/w.
  - Per core: qT/kT projections emitted in head-transposed layout (d x tok),
    rotary applied via a pair-swap permutation matmul + cos/sin elementwise;
    v projected then PE-transposed to natural (tok x d) layout.
  - Attention per (batch, local head) with scores computed TRANSPOSED
    (kv on partitions, q on free axis): softmax needs no cross-partition max
    (scores are bounded, exp(-1e9) underflows to 0 for the causal mask), the
    denominator comes from a ones-column matmul, and probs feed the PV matmul
    directly without any transposes.
  - Local head outputs are normalized, cast to bf16, AllGather'ed across the
    8 cores (4 chunks, overlapped with the second batch's attention + wo
    matmuls), then each core computes its 512-wide column shard of out @ wo.T
    in bf16.
  - Projections/attention matmuls run as float32r (fp22 reduced precision,
    full PE rate at free-dim >= 256) with fp32 PSUM accumulation.

Outputs returned per core and re-assembled (transpose/concat) on the host.
"""

import math
import os
import sys

import numpy as np

for _p in ("/opt/trn_rl_repo", "/root/.axon_site/_ro/trn_rl_repo"):
    if os.path.isdir(_p) and _p not in sys.path:
        sys.path.insert(0, _p)

N_CORES = 8
FULL = dict(D=4096, S=2048, B=2, H=32)

_PROG_CACHE = {}


def build_program(D, S, B, H):
    """Build + compile the SPMD Bass program. Returns (nc, meta)."""
    import concourse.bass as bass
    import concourse.tile as tile
    from concourse import bacc, mybir

    f32 = mybir.dt.float32
    f32r = mybir.dt.float32r
    bf16 = mybir.dt.bfloat16
    Exp = mybir.ActivationFunctionType.Exp

    NC = N_CORES
    TOK = B * S
    HL = H // NC          # local heads
    DL = HL * 128         # local width of q/k/v/out shards
    assert H * 128 == D
    ND = D // 128         # contraction tiles
    NM = DL // 128        # local dout tiles
    TOKC = 512            # token chunk (matmul free dim)
    NTC = TOK // TOKC
    QC = 512              # q chunk in attention
    NQC = S // QC
    NKI = S // 128        # kv tiles per head
    XG = min(8, ND)       # din tiles per x-slab DMA
    NG = ND // XG
    AGC = min(512, TOK)   # allgather chunk width (tokens)
    NAG = TOK // AGC
    NB = QC // 128        # causal band masks
    WCH = 8               # weight-slab dt tiles per DMA chunk
    scale = 1.0 / math.sqrt(128.0)

    nc = bacc.Bacc("TRN2", target_bir_lowering=False, debug=False, num_devices=NC)

    def din(name, shape, dt=f32):
        return nc.dram_tensor(name, list(shape), dt, kind="ExternalInput").ap()

    xT = din("xT", (D, TOK))
    wT = {p: din(f"w{p}T", (D, DL)) for p in ("q", "k", "v")}
    woT = din("woT", (D, DL), bf16)
    cmat = din("cmat", (128, S))
    smat = din("smat", (128, S))
    consts = din("consts", (128, 512))
    onesb = din("onesb", (1, 128), bf16)
    maskb = din("maskb", (128, NB * 512))

    out_t = nc.dram_tensor("out_t", [DL, TOK], f32, kind="ExternalOutput").ap()
    k_t = nc.dram_tensor("k_t", [DL, TOK], f32, kind="ExternalOutput").ap()
    v_n = nc.dram_tensor("v_n", [TOK, DL], f32, kind="ExternalOutput").ap()
    q_t = nc.dram_tensor("q_t", [DL, TOK], f32).ap()
    ag_in = [nc.dram_tensor(f"ag_in{j}", [DL, AGC], bf16).ap() for j in range(NAG)]
    ag_out = [
        nc.dram_tensor(f"ag_out{j}", [D, AGC], bf16, addr_space="Shared").ap()
        for j in range(NAG)
    ]

    xT_r = xT.bitcast(f32r).rearrange("(dt p) t -> p dt t", p=128)

    def load_w_slab(pool, wap, dtype, tag):
        """Weight slab [128, ND*DL], chunked so early matmuls unblock fast."""
        t = pool.tile([128, ND * DL], dtype, tag=tag, name=f"{tag}_sb")
        tr = t[:].rearrange("p (dt j) -> p dt j", dt=ND)
        wr = wap.bitcast(dtype) if dtype == f32r else wap
        wr = wr.rearrange("(dt p) j -> p dt j", p=128)
        for c0 in range(0, ND, WCH):
            c1 = min(c0 + WCH, ND)
            nc.sync.dma_start(tr[:, c0:c1], wr[:, c0:c1])
        return t

    with tile.TileContext(nc) as tc:
        with tc.tile_pool(name="const", bufs=1) as cp:
            ccr = cp.tile([128, 129], f32r)
            nc.sync.dma_start(ccr[:], consts.bitcast(f32r)[:, 0:129])
            pt_r = ccr[:, 0:128]
            ones_col_r = ccr[:, 128:129]
            cc = cp.tile([128, 257], f32)
            nc.sync.dma_start(cc[:], consts[:, 129:386])
            ident_sb = cc[:, 0:128]
            ones_row = cc[0:1, 128:256]
            ones_col_f = cc[:, 256:257]
            onesb_sb = cp.tile([1, 128], bf16)
            nc.sync.dma_start(onesb_sb[:], onesb)
            mb_sb = cp.tile([128, NB * 512], f32)
            nc.sync.dma_start(mb_sb[:], maskb)

            # ---------- P1: Q,K projections + rotary (transposed layout) ----
            with tc.tile_pool(name="p1c", bufs=1) as c1p, \
                 tc.tile_pool(name="p1w", bufs=1) as wp, \
                 tc.tile_pool(name="p1x", bufs=2) as xp, \
                 tc.tile_pool(name="p1o", bufs=3) as op_, \
                 tc.tile_pool(name="p1ps", bufs=2 * NM, space="PSUM") as pp:
                cmat_sb = c1p.tile([128, S], f32)
                nc.sync.dma_start(cmat_sb[:], cmat)
                smat_sb = c1p.tile([128, S], f32)
                nc.sync.dma_start(smat_sb[:], smat)
                # first x slab before the weight slabs: matmuls need both,
                # and this orders the DMA queue so dt=0 work unblocks early
                xg0 = xp.tile([128, XG * TOKC], f32r, tag="xg", name="xg_0_0")
                nc.sync.dma_start(
                    xg0[:].rearrange("p (i t) -> p i t", i=XG),
                    xT_r[:, 0:XG, 0:TOKC],
                )
                w_sb = {}
                w_tr = {}
                for p in ("q", "k"):
                    t = wp.tile([128, ND * DL], f32r, tag=f"w{p}", name=f"w{p}_sb")
                    w_sb[p] = t
                    w_tr[p] = (t[:].rearrange("p (dt j) -> p dt j", dt=ND),
                               wT[p].bitcast(f32r).rearrange("(dt p) j -> p dt j", p=128))
                for c0 in range(0, ND, WCH):
                    c1 = min(c0 + WCH, ND)
                    for p in ("q", "k"):
                        tr, wr = w_tr[p]
                        nc.sync.dma_start(tr[:, c0:c1], wr[:, c0:c1])
                projs = (("q", q_t), ("k", k_t))
                for tokc in range(NTC):
                    scol = (tokc * TOKC) % S
                    xg = []
                    for g in range(NG):
                        if tokc == 0 and g == 0:
                            xg.append(xg0)
                            continue
                        t = xp.tile([128, XG * TOKC], f32r, tag="xg", name=f"xg_{tokc}_{g}")
                        nc.sync.dma_start(
                            t[:].rearrange("p (i t) -> p i t", i=XG),
                            xT_r[:, g * XG:(g + 1) * XG, tokc * TOKC:(tokc + 1) * TOKC],
                        )
                        xg.append(t)
                    psums = {}
                    for pname, _ in projs:
                        for m in range(NM):
                            psums[(pname, m)] = pp.tile([128, TOKC], f32, tag="pr", name=f"ps_{tokc}_{pname}{m}")
                    for dt in range(ND):
                        g, i = divmod(dt, XG)
                        rhs = xg[g][:, i * TOKC:(i + 1) * TOKC]
                        for pname, _ in projs:
                            for m in range(NM):
                                nc.tensor.matmul(
                                    psums[(pname, m)][:],
                                    w_sb[pname][:, dt * DL + m * 128: dt * DL + (m + 1) * 128],
                                    rhs,
                                    start=(dt == 0), stop=(dt == ND - 1),
                                )
                    for pname, dst in projs:
                        for m in range(NM):
                            ps = psums[(pname, m)]
                            raw = op_.tile([128, TOKC], f32r, tag="raw")
                            nc.scalar.copy(raw[:], ps[:])
                            psw = pp.tile([128, TOKC], f32, tag="pr")
                            nc.tensor.matmul(psw[:], pt_r, raw[:], start=True, stop=True)
                            t2 = op_.tile([128, TOKC], f32, tag="t2")
                            nc.vector.tensor_mul(t2[:], psw[:], smat_sb[:, scol:scol + TOKC])
                            t1 = op_.tile([128, TOKC], f32, tag="t1")
                            nc.vector.tensor_mul(t1[:], raw[:].bitcast(f32), cmat_sb[:, scol:scol + TOKC])
                            nc.vector.tensor_add(t1[:], t1[:], t2[:])
                            nc.sync.dma_start(
                                dst[m * 128:(m + 1) * 128, tokc * TOKC:(tokc + 1) * TOKC],
                                t1[:],
                            )

            # ---------- P2+P3+P4: V projection, attention, AllGather, wo ----
            # One nested scope tree so the v-projection matmuls fill batch-0
            # attention slack and the wo matmuls fill batch-1 slack.
            with tc.tile_pool(name="p3qkv", bufs=2) as qp, \
                 tc.tile_pool(name="p3pr", bufs=min(14, NKI + 2)) as prp, \
                 tc.tile_pool(name="p3o", bufs=3) as op_, \
                 tc.tile_pool(name="p3a", bufs=2) as accp, \
                 tc.tile_pool(name="p3ps", bufs=2, space="PSUM") as psc, \
                 tc.tile_pool(name="p3po", bufs=1, space="PSUM") as pso, \
                 tc.tile_pool(name="p3pl", bufs=1, space="PSUM") as psl:
                v_n_r = v_n.bitcast(f32r).rearrange("(t2 p) j -> p t2 j", p=128)
                NQH = (NQC + 1) // 2

                def emit_v_chunks(wv_sb, xp, vop, pp, tokcs):
                    for tokc in tokcs:
                        xg = []
                        for g in range(NG):
                            t = xp.tile([128, XG * TOKC], f32r, tag="xg", name=f"xg2_{tokc}_{g}")
                            nc.sync.dma_start(
                                t[:].rearrange("p (i t) -> p i t", i=XG),
                                xT_r[:, g * XG:(g + 1) * XG, tokc * TOKC:(tokc + 1) * TOKC],
                            )
                            xg.append(t)
                        psums = [pp.tile([128, TOKC], f32, tag="pv", name=f"psv_{tokc}_{m}") for m in range(NM)]
                        for dt in range(ND):
                            g, i = divmod(dt, XG)
                            rhs = xg[g][:, i * TOKC:(i + 1) * TOKC]
                            for m in range(NM):
                                nc.tensor.matmul(
                                    psums[m][:],
                                    wv_sb[:, dt * DL + m * 128: dt * DL + (m + 1) * 128],
                                    rhs,
                                    start=(dt == 0), stop=(dt == ND - 1),
                                )
                        vT_sb = []
                        for m in range(NM):
                            t = vop.tile([128, TOKC], f32, tag="vT", name=f"vT_{tokc}_{m}")
                            nc.scalar.copy(t[:], psums[m][:])
                            vT_sb.append(t)
                        for tt in range(TOKC // 128):
                            ptr = pp.tile([128, NM * 128], f32, tag="pv", name=f"ptr_{tokc}_{tt}")
                            for m in range(NM):
                                nc.tensor.transpose(
                                    ptr[:, m * 128:(m + 1) * 128],
                                    vT_sb[m][:, tt * 128:(tt + 1) * 128],
                                    ident_sb,
                                )
                            vn = vop.tile([128, NM * 128], f32, tag="vn")
                            nc.vector.tensor_copy(vn[:], ptr[:])
                            nc.sync.dma_start(
                                v_n[tokc * TOKC + tt * 128: tokc * TOKC + (tt + 1) * 128, :],
                                vn[:],
                            )

                def emit_attn_batch(b):
                    for qch in range(NQH):
                        qcs = [qc for qc in (2 * qch, 2 * qch + 1) if qc < NQC]
                        nki_max = min(4 * (qcs[-1] + 1), NKI)
                        for hl in range(HL):
                            qT_sb = qp.tile([128, len(qcs) * QC], f32r, tag="qT",
                                            name=f"qT_{b}_{qch}_{hl}")
                            nc.sync.dma_start(
                                qT_sb[:],
                                q_t.bitcast(f32r)[hl * 128:(hl + 1) * 128,
                                                  b * S + qcs[0] * QC: b * S + (qcs[-1] + 1) * QC])
                            kT_sb = qp.tile([128, nki_max * 128], f32r, tag="kT",
                                            name=f"kT_{b}_{qch}_{hl}")
                            nc.sync.dma_start(
                                kT_sb[:],
                                k_t.bitcast(f32r)[hl * 128:(hl + 1) * 128,
                                                  b * S: b * S + nki_max * 128])
                            v_sb = qp.tile([128, nki_max * 128], f32r, tag="v",
                                           name=f"v_{b}_{qch}_{hl}")
                            nc.sync.dma_start(
                                v_sb[:].rearrange("p (ki j) -> p ki j", ki=nki_max),
                                v_n_r[:, b * NKI: b * NKI + nki_max, hl * 128:(hl + 1) * 128],
                            )
                            for qc in qcs:
                                nki_q = min(4 * (qc + 1), NKI)
                                qoff = (qc - qcs[0]) * QC
                                probs = []
                                acc = accp.tile([128, QC], f32, tag="acc")
                                for ki in range(nki_q):
                                    p_sc = psc.tile([128, QC], f32, tag="sc")
                                    nc.tensor.matmul(
                                        p_sc[:],
                                        kT_sb[:, ki * 128:(ki + 1) * 128],
                                        qT_sb[:, qoff:qoff + QC],
                                        start=True, stop=True,
                                    )
                                    j = ki - (qc * QC) // 128
                                    if j >= 0:
                                        nc.vector.tensor_add(p_sc[:], p_sc[:], mb_sb[:, j * 512:(j + 1) * 512])
                                    pt_ = prp.tile([128, QC], f32r, tag="probs",
                                                   name=f"probs_{b}_{hl}_{qc}_{ki}")
                                    nc.scalar.activation(pt_[:], p_sc[:], Exp, scale=scale)
                                    if ki == 0:
                                        nc.vector.tensor_copy(acc[:], pt_[:].bitcast(f32))
                                    else:
                                        nc.vector.tensor_add(acc[:], acc[:], pt_[:].bitcast(f32))
                                    probs.append(pt_)
                                p_out = pso.tile([128, QC], f32, tag="po")
                                p_l = psl.tile([1, QC], f32, tag="pl")
                                for ki in range(nki_q):
                                    nc.tensor.matmul(
                                        p_out[:],
                                        v_sb[:, ki * 128:(ki + 1) * 128],
                                        probs[ki][:],
                                        start=(ki == 0), stop=(ki == nki_q - 1),
                                    )
                                nc.tensor.matmul(p_l[:], ones_col_f, acc[:], start=True, stop=True)
                                r_sb = op_.tile([1, QC], bf16, tag="r")
                                with nc.allow_low_precision(reason="1/l broadcast in bf16 is intentional"):
                                    nc.vector.reciprocal(r_sb[:], p_l[:])
                                p_r = psc.tile([128, QC], f32, tag="sc")
                                nc.tensor.matmul(p_r[:], onesb_sb[:], r_sb[:], start=True, stop=True)
                                rbc = op_.tile([128, QC], f32, tag="rbc")
                                nc.vector.tensor_copy(rbc[:], p_r[:])
                                attn = op_.tile([128, QC], bf16, tag="attn")
                                nc.vector.tensor_mul(attn[:], p_out[:], rbc[:])
                                tok0 = b * S + qc * QC
                                j = tok0 // AGC
                                col = tok0 % AGC
                                nc.sync.dma_start(
                                    ag_in[j][hl * 128:(hl + 1) * 128, col:col + QC], attn[:]
                                )
                                if hl == HL - 1:
                                    for jj in range(NAG):
                                        last_b = min(B - 1, (AGC * (jj + 1) - 1) // S)
                                        last_qc = min(NQC - 1, (min(AGC * (jj + 1), (last_b + 1) * S) - 1 - last_b * S) // QC)
                                        if last_b == b and last_qc == qc:
                                            nc.gpsimd.collective_compute(
                                                "AllGather",
                                                mybir.AluOpType.bypass,
                                                replica_groups=[list(range(NC))],
                                                ins=[ag_in[jj].opt()],
                                                outs=[ag_out[jj].opt()],
                                            )

                with tc.tile_pool(name="p2w", bufs=1) as wpv, \
                     tc.tile_pool(name="p2x", bufs=2) as xpv, \
                     tc.tile_pool(name="p2o", bufs=NM) as vop, \
                     tc.tile_pool(name="p2ps", bufs=max(NM, TOKC // 128), space="PSUM") as ppv:
                    wv_sb = load_w_slab(wpv, wT["v"], f32r, "wv")
                    emit_v_chunks(wv_sb, xpv, vop, ppv, range(0, NTC // B))
                    emit_attn_batch(0)
                    emit_v_chunks(wv_sb, xpv, vop, ppv, range(NTC // B, NTC))

                # ---------- P4 (pools take over the v-projection's space) ----
                with tc.tile_pool(name="p4w", bufs=1) as wp4, \
                     tc.tile_pool(name="p4x", bufs=5) as xp4, \
                     tc.tile_pool(name="p4o", bufs=2) as op4, \
                     tc.tile_pool(name="p4ps", bufs=min(NM, 4), space="PSUM") as pp4:
                    wo_sb = load_w_slab(wp4, woT, bf16, "wo")
                    for bb in range(1, B):
                        emit_attn_batch(bb)
                    ag_out_r = [a.rearrange("(dt p) t -> p dt t", p=128) for a in ag_out]
                    for tokc in range(NTC):
                        j = (tokc * TOKC) // AGC
                        col = (tokc * TOKC) % AGC
                        xg = []
                        for g in range(NG):
                            t = xp4.tile([128, XG * TOKC], bf16, tag="ag", name=f"agx_{tokc}_{g}")
                            nc.sync.dma_start(
                                t[:].rearrange("p (i t) -> p i t", i=XG),
                                ag_out_r[j][:, g * XG:(g + 1) * XG, col:col + TOKC],
                            )
                            xg.append(t)
                        psums = {m: pp4.tile([128, TOKC], f32, tag="pw", name=f"psw_{tokc}_{m}") for m in range(NM)}
                        for dt in range(ND):
                            g, i = divmod(dt, XG)
                            rhs = xg[g][:, i * TOKC:(i + 1) * TOKC]
                            for m in range(NM):
                                nc.tensor.matmul(
                                    psums[m][:],
                                    wo_sb[:, dt * DL + m * 128: dt * DL + (m + 1) * 128],
                                    rhs,
                                    start=(dt == 0), stop=(dt == ND - 1),
                                )
                        for m in range(NM):
                            ow = op4.tile([128, TOKC], f32, tag="ow")
                            nc.scalar.copy(ow[:], psums[m][:])
                            nc.sync.dma_start(
                                out_t[m * 128:(m + 1) * 128, tokc * TOKC:(tokc + 1) * TOKC],
                                ow[:],
                            )

    nc.compile()
    meta = dict(D=D, S=S, B=B, H=H, HL=HL, DL=DL, NB=NB)
    return nc, meta


def make_inputs(x, freqs_cis, wq, wk, wv, wo, D, S, B, H):
    """Host-side sharding: build the per-core in_maps."""
    import ml_dtypes

    NC = N_CORES
    TOK = B * S
    HL = H // NC
    DL = HL * 128
    NB = 4

    xT = np.ascontiguousarray(x.reshape(TOK, D).T)

    fc = np.asarray(freqs_cis, np.float32)           # (S, 64, 2)
    cmat = np.repeat(np.ascontiguousarray(fc[:, :, 0].T), 2, axis=0).astype(np.float32)
    smat = np.repeat(np.ascontiguousarray(fc[:, :, 1].T), 2, axis=0).astype(np.float32)

    P = np.zeros((128, 128), np.float32)
    idx = np.arange(64)
    P[2 * idx, 2 * idx + 1] = -1.0
    P[2 * idx + 1, 2 * idx] = 1.0
    consts = np.zeros((128, 512), np.float32)
    consts[:, 0:128] = P.T
    consts[:, 128] = 1.0                     # ones_col (f32r view)
    consts[:, 129:257] = np.eye(128, dtype=np.float32)
    consts[0, 257:385] = 1.0                 # ones_row
    consts[:, 385] = 1.0                     # ones_col (f32)

    maskb = np.zeros((128, NB * 512), np.float32)
    p = np.arange(128)[:, None]
    f = np.arange(512)[None, :]
    for j in range(NB):
        maskb[:, j * 512:(j + 1) * 512] = np.where(128 * j + p <= f, 0.0, -1e9)

    in_maps = []
    for c in range(NC):
        sl = slice(c * DL, (c + 1) * DL)
        in_maps.append({
            "xT": xT,
            "wqT": np.ascontiguousarray(wq[sl, :].T),
            "wkT": np.ascontiguousarray(wk[sl, :].T),
            "wvT": np.ascontiguousarray(wv[sl, :].T),
            "woT": np.ascontiguousarray(wo[sl, :].T).astype(ml_dtypes.bfloat16),
            "cmat": cmat,
            "smat": smat,
            "consts": consts,
            "onesb": np.ones((1, 128), ml_dtypes.bfloat16),
            "maskb": maskb,
        })
    return in_maps


def assemble_outputs(results, D, S, B, H):
    """Gather per-core outputs into (out, new_hidden)."""
    NC = N_CORES
    TOK = B * S
    HL = H // NC
    DL = HL * 128

    out = np.empty((TOK, D), np.float32)
    k_full = np.empty((B, S, H, 128), np.float32)
    v_full = np.empty((B, S, H, 128), np.float32)
    for c in range(NC):
        r = results[c]
        out[:, c * DL:(c + 1) * DL] = np.asarray(r["out_t"], np.float32).T
        kt = np.asarray(r["k_t"], np.float32).reshape(HL, 128, B, S).transpose(2, 3, 0, 1)
        k_full[:, :, c * HL:(c + 1) * HL, :] = kt
        v_full[:, :, c * HL:(c + 1) * HL, :] = np.asarray(r["v_n"], np.float32).reshape(B, S, HL, 128)
    out = out.reshape(B, S, D)
    new_hidden = np.stack([k_full, v_full], axis=0)
    return out, new_hidden


def _reference_fallback(x, freqs_cis, mask, hidden_state, wq, wk, wv, wo, start_pos):
    """Pure-numpy mirror of the reference for inputs the fast path can't take."""
    bsz, seqlen, dim = x.shape
    h, hd = dim // 128, 128
    xt = x.reshape(bsz * seqlen, dim).astype(np.float32)
    q = (xt @ wq.T).reshape(bsz, seqlen, h, hd)
    k = (xt @ wk.T).reshape(bsz, seqlen, h, hd)
    v = (xt @ wv.T).reshape(bsz, seqlen, h, hd)
    fc = np.asarray(freqs_cis, np.float32)

    def rot(z):
        zz = z.reshape(bsz, seqlen, h, hd // 2, 2)
        cos = fc[:, :, 0][None, :, None, :]
        sin = fc[:, :, 1][None, :, None, :]
        r = zz[..., 0] * cos - zz[..., 1] * sin
        im = zz[..., 0] * sin + zz[..., 1] * cos
        return np.stack([r, im], -1).reshape(bsz, seqlen, h, hd).astype(np.float32)

    qr, kr = rot(q), rot(k)
    new_k = np.array(hidden_state[0])
    new_v = np.array(hidden_state[1])
    new_k[:bsz, start_pos:start_pos + seqlen] = kr
    new_v[:bsz, start_pos:start_pos + seqlen] = v
    keys = new_k[:bsz, :start_pos + seqlen]
    values = new_v[:bsz, :start_pos + seqlen]
    out = np.zeros((bsz, seqlen, h, hd), np.float32)
    m2 = np.asarray(mask, np.float32)[0, 0]
    for b in range(bsz):
        for hh in range(h):
            sc = (qr[b, :, hh, :] @ keys[b, :, hh, :].T) / math.sqrt(hd) + m2
            sc = sc - sc.max(-1, keepdims=True)
            p = np.exp(sc)
            p = p / p.sum(-1, keepdims=True)
            out[b, :, hh, :] = p @ values[b, :, hh, :]
    return (out.reshape(bsz, seqlen, h * hd) @ wo.T,
            np.stack([new_k, new_v], 0))


def _is_causal_mask(mask, S):
    m = np.asarray(mask, np.float32).reshape(S, S)
    expect = np.triu(np.full((S, S), -1e9, np.float32), k=1)
    return m.shape == expect.shape and np.array_equal(m, expect)


def run_on_device(in_maps, prog_key=None, trace=False):
    from concourse.bass_utils import run_bass_kernel_spmd

    key = prog_key or tuple(sorted(FULL.items()))
    if key not in _PROG_CACHE:
        _PROG_CACHE[key] = build_program(**dict(key))
    nc, meta = _PROG_CACHE[key]
    res = run_bass_kernel_spmd(
        nc, in_maps, core_ids=list(range(N_CORES)), trace=trace
    )
    return res, meta


def kernel(x, freqs_cis, mask, hidden_state, wq, wk, wv, wo, start_pos):
    x = np.asarray(x, np.float32)
    freqs_cis = np.asarray(freqs_cis, np.float32)
    wq = np.asarray(wq, np.float32)
    wk = np.asarray(wk, np.float32)
    wv = np.asarray(wv, np.float32)
    wo = np.asarray(wo, np.float32)
    D, S, B, H = FULL["D"], FULL["S"], FULL["B"], FULL["H"]

    sp = int(start_pos)
    if (sp != 0 or x.shape != (B, S, D) or not _is_causal_mask(mask, S)):
        return _reference_fallback(
            x, freqs_cis, np.asarray(mask), np.asarray(hidden_state),
            wq, wk, wv, wo, sp)

    in_maps = make_inputs(x, freqs_cis, wq, wk, wv, wo, D, S, B, H)
    res, _meta = run_on_device(in_maps)
    out, new_hidden = assemble_outputs(res.results, D, S, B, H)
    return out, new_hidden


# revision 22
# speedup vs baseline: 1.0809x; 1.0361x over previous
"""Tensor-parallel causal attention block (dense transformer) on 8 TRN2 NeuronCores.

Strategy (tensor parallel over heads, 4 heads / core):
  - Host pre-transposes x -> xT (din on rows) and the per-core weight shards
    (wq/wk/wv column shards, wo row shard), so every on-device matmul has its
    contraction dim on SBUF partitions with no on-device transposes of x/w.
  - Per core: qT/kT projections emitted in head-transposed layout (d x tok),
    rotary applied via a pair-swap permutation matmul + cos/sin elementwise;
    v projected then PE-transposed to natural (tok x d) layout.
  - Attention per (batch, local head) with scores computed TRANSPOSED
    (kv on partitions, q on free axis): softmax needs no cross-partition max
    (scores are bounded, exp(-1e9) underflows to 0 for the causal mask), the
    denominator comes from a ones-column matmul, and probs feed the PV matmul
    directly without any transposes.
  - Local head outputs are normalized, cast to bf16, AllGather'ed across the
    8 cores (4 chunks, overlapped with the second batch's attention + wo
    matmuls), then each core computes its 512-wide column shard of out @ wo.T
    in bf16.
  - Projections/attention matmuls run as float32r (fp22 reduced precision,
    full PE rate at free-dim >= 256) with fp32 PSUM accumulation.

Outputs returned per core and re-assembled (transpose/concat) on the host.
"""

import math
import os
import sys

import numpy as np

for _p in ("/opt/trn_rl_repo", "/root/.axon_site/_ro/trn_rl_repo"):
    if os.path.isdir(_p) and _p not in sys.path:
        sys.path.insert(0, _p)

N_CORES = 8
FULL = dict(D=4096, S=2048, B=2, H=32)

_PROG_CACHE = {}


def build_program(D, S, B, H):
    """Build + compile the SPMD Bass program. Returns (nc, meta)."""
    import concourse.bass as bass
    import concourse.tile as tile
    from concourse import bacc, mybir

    f32 = mybir.dt.float32
    f32r = mybir.dt.float32r
    bf16 = mybir.dt.bfloat16
    Exp = mybir.ActivationFunctionType.Exp

    NC = N_CORES
    TOK = B * S
    HL = H // NC          # local heads
    DL = HL * 128         # local width of q/k/v/out shards
    assert H * 128 == D
    ND = D // 128         # contraction tiles
    NM = DL // 128        # local dout tiles
    TOKC = 512            # token chunk (matmul free dim)
    NTC = TOK // TOKC
    QC = 512              # q chunk in attention
    NQC = S // QC
    NKI = S // 128        # kv tiles per head
    XG = min(8, ND)       # din tiles per x-slab DMA
    NG = ND // XG
    AGC = min(512, TOK)   # allgather chunk width (tokens)
    NAG = TOK // AGC
    NB = QC // 128        # causal band masks
    WCH = 8               # weight-slab dt tiles per DMA chunk
    scale = 1.0 / math.sqrt(128.0)

    nc = bacc.Bacc("TRN2", target_bir_lowering=False, debug=False, num_devices=NC)

    def din(name, shape, dt=f32):
        return nc.dram_tensor(name, list(shape), dt, kind="ExternalInput").ap()

    xT = din("xT", (D, TOK))
    wT = {p: din(f"w{p}T", (D, DL)) for p in ("q", "k", "v")}
    woT = din("woT", (D, DL), bf16)
    cmat = din("cmat", (128, S))
    smat = din("smat", (128, S))
    consts = din("consts", (128, 512))
    onesb = din("onesb", (1, 128), bf16)
    maskb = din("maskb", (128, NB * 512))

    out_t = nc.dram_tensor("out_t", [DL, TOK], f32, kind="ExternalOutput").ap()
    k_t = nc.dram_tensor("k_t", [DL, TOK], f32, kind="ExternalOutput").ap()
    v_n = nc.dram_tensor("v_n", [TOK, DL], f32, kind="ExternalOutput").ap()
    q_t = nc.dram_tensor("q_t", [DL, TOK], f32).ap()
    ag_in = [nc.dram_tensor(f"ag_in{j}", [DL, AGC], bf16).ap() for j in range(NAG)]
    ag_out = [
        nc.dram_tensor(f"ag_out{j}", [D, AGC], bf16, addr_space="Shared").ap()
        for j in range(NAG)
    ]

    xT_r = xT.bitcast(f32r).rearrange("(dt p) t -> p dt t", p=128)

    def load_w_slab(pool, wap, dtype, tag):
        """Weight slab [128, ND*DL], chunked so early matmuls unblock fast."""
        t = pool.tile([128, ND * DL], dtype, tag=tag, name=f"{tag}_sb")
        tr = t[:].rearrange("p (dt j) -> p dt j", dt=ND)
        wr = wap.bitcast(dtype) if dtype == f32r else wap
        wr = wr.rearrange("(dt p) j -> p dt j", p=128)
        for c0 in range(0, ND, WCH):
            c1 = min(c0 + WCH, ND)
            nc.sync.dma_start(tr[:, c0:c1], wr[:, c0:c1])
        return t

    with tile.TileContext(nc) as tc:
        with tc.tile_pool(name="const", bufs=1) as cp:
            ccr = cp.tile([128, 129], f32r)
            nc.sync.dma_start(ccr[:], consts.bitcast(f32r)[:, 0:129])
            pt_r = ccr[:, 0:128]
            ones_col_r = ccr[:, 128:129]
            cc = cp.tile([128, 257], f32)
            nc.sync.dma_start(cc[:], consts[:, 129:386])
            ident_sb = cc[:, 0:128]
            ones_row = cc[0:1, 128:256]
            ones_col_f = cc[:, 256:257]
            onesb_sb = cp.tile([1, 128], bf16)
            nc.sync.dma_start(onesb_sb[:], onesb)
            mb_sb = cp.tile([128, NB * 512], f32)
            nc.sync.dma_start(mb_sb[:], maskb)

            # ---------- P1: Q,K projections + rotary (transposed layout) ----
            with tc.tile_pool(name="p1c", bufs=1) as c1p, \
                 tc.tile_pool(name="p1w", bufs=1) as wp, \
                 tc.tile_pool(name="p1x", bufs=2) as xp, \
                 tc.tile_pool(name="p1o", bufs=3) as op_, \
                 tc.tile_pool(name="p1ps", bufs=2 * NM, space="PSUM") as pp:
                cmat_sb = c1p.tile([128, S], f32)
                nc.sync.dma_start(cmat_sb[:], cmat)
                smat_sb = c1p.tile([128, S], f32)
                nc.sync.dma_start(smat_sb[:], smat)
                # first x slab before the weight slabs: matmuls need both,
                # and this orders the DMA queue so dt=0 work unblocks early
                xg0 = xp.tile([128, XG * TOKC], f32r, tag="xg", name="xg_0_0")
                nc.sync.dma_start(
                    xg0[:].rearrange("p (i t) -> p i t", i=XG),
                    xT_r[:, 0:XG, 0:TOKC],
                )
                w_sb = {}
                w_tr = {}
                for p in ("q", "k"):
                    t = wp.tile([128, ND * DL], f32r, tag=f"w{p}", name=f"w{p}_sb")
                    w_sb[p] = t
                    w_tr[p] = (t[:].rearrange("p (dt j) -> p dt j", dt=ND),
                               wT[p].bitcast(f32r).rearrange("(dt p) j -> p dt j", p=128))
                for c0 in range(0, ND, WCH):
                    c1 = min(c0 + WCH, ND)
                    for p in ("q", "k"):
                        tr, wr = w_tr[p]
                        nc.sync.dma_start(tr[:, c0:c1], wr[:, c0:c1])
                projs = (("q", q_t), ("k", k_t))
                for tokc in range(NTC):
                    scol = (tokc * TOKC) % S
                    xg = []
                    for g in range(NG):
                        if tokc == 0 and g == 0:
                            xg.append(xg0)
                            continue
                        t = xp.tile([128, XG * TOKC], f32r, tag="xg", name=f"xg_{tokc}_{g}")
                        nc.sync.dma_start(
                            t[:].rearrange("p (i t) -> p i t", i=XG),
                            xT_r[:, g * XG:(g + 1) * XG, tokc * TOKC:(tokc + 1) * TOKC],
                        )
                        xg.append(t)
                    psums = {}
                    for pname, _ in projs:
                        for m in range(NM):
                            psums[(pname, m)] = pp.tile([128, TOKC], f32, tag="pr", name=f"ps_{tokc}_{pname}{m}")
                    for dt in range(ND):
                        g, i = divmod(dt, XG)
                        rhs = xg[g][:, i * TOKC:(i + 1) * TOKC]
                        for pname, _ in projs:
                            for m in range(NM):
                                nc.tensor.matmul(
                                    psums[(pname, m)][:],
                                    w_sb[pname][:, dt * DL + m * 128: dt * DL + (m + 1) * 128],
                                    rhs,
                                    start=(dt == 0), stop=(dt == ND - 1),
                                )
                    for pname, dst in projs:
                        for m in range(NM):
                            ps = psums[(pname, m)]
                            raw = op_.tile([128, TOKC], f32r, tag="raw")
                            nc.scalar.copy(raw[:], ps[:])
                            # swap matmul overwrites the drained projection bank
                            # in place: halves P1 psum-slot churn per chunk
                            psw = ps
                            nc.tensor.matmul(psw[:], pt_r, raw[:], start=True, stop=True)
                            t2 = op_.tile([128, TOKC], f32, tag="t2")
                            nc.vector.tensor_mul(t2[:], psw[:], smat_sb[:, scol:scol + TOKC])
                            t1 = op_.tile([128, TOKC], f32, tag="t1")
                            nc.vector.tensor_mul(t1[:], raw[:].bitcast(f32), cmat_sb[:, scol:scol + TOKC])
                            nc.vector.tensor_add(t1[:], t1[:], t2[:])
                            nc.sync.dma_start(
                                dst[m * 128:(m + 1) * 128, tokc * TOKC:(tokc + 1) * TOKC],
                                t1[:],
                            )

            # ---------- P2+P3+P4: V projection, attention, AllGather, wo ----
            # One nested scope tree so the v-projection matmuls fill batch-0
            # attention slack and the wo matmuls fill batch-1 slack.
            with tc.tile_pool(name="p3qkv", bufs=2) as qp, \
                 tc.tile_pool(name="p3pr", bufs=min(14, NKI + 2)) as prp, \
                 tc.tile_pool(name="p3o", bufs=3) as op_, \
                 tc.tile_pool(name="p3a", bufs=2) as accp, \
                 tc.tile_pool(name="p3ps", bufs=2, space="PSUM") as psc, \
                 tc.tile_pool(name="p3po", bufs=1, space="PSUM") as pso, \
                 tc.tile_pool(name="p3pl", bufs=1, space="PSUM") as psl:
                v_n_r = v_n.bitcast(f32r).rearrange("(t2 p) j -> p t2 j", p=128)
                NQH = (NQC + 1) // 2

                def emit_v_chunks(wv_sb, xp, vop, pp, tokcs):
                    for tokc in tokcs:
                        xg = []
                        for g in range(NG):
                            t = xp.tile([128, XG * TOKC], f32r, tag="xg", name=f"xg2_{tokc}_{g}")
                            nc.sync.dma_start(
                                t[:].rearrange("p (i t) -> p i t", i=XG),
                                xT_r[:, g * XG:(g + 1) * XG, tokc * TOKC:(tokc + 1) * TOKC],
                            )
                            xg.append(t)
                        psums = [pp.tile([128, TOKC], f32, tag="pv", name=f"psv_{tokc}_{m}") for m in range(NM)]
                        for dt in range(ND):
                            g, i = divmod(dt, XG)
                            rhs = xg[g][:, i * TOKC:(i + 1) * TOKC]
                            for m in range(NM):
                                nc.tensor.matmul(
                                    psums[m][:],
                                    wv_sb[:, dt * DL + m * 128: dt * DL + (m + 1) * 128],
                                    rhs,
                                    start=(dt == 0), stop=(dt == ND - 1),
                                )
                        vT_sb = []
                        for m in range(NM):
                            t = vop.tile([128, TOKC], f32, tag="vT", name=f"vT_{tokc}_{m}")
                            nc.scalar.copy(t[:], psums[m][:])
                            vT_sb.append(t)
                        for tt in range(TOKC // 128):
                            ptr = pp.tile([128, NM * 128], f32, tag="pv", name=f"ptr_{tokc}_{tt}")
                            for m in range(NM):
                                nc.tensor.transpose(
                                    ptr[:, m * 128:(m + 1) * 128],
                                    vT_sb[m][:, tt * 128:(tt + 1) * 128],
                                    ident_sb,
                                )
                            vn = vop.tile([128, NM * 128], f32, tag="vn")
                            nc.vector.tensor_copy(vn[:], ptr[:])
                            nc.sync.dma_start(
                                v_n[tokc * TOKC + tt * 128: tokc * TOKC + (tt + 1) * 128, :],
                                vn[:],
                            )

                def emit_attn_batch(b):
                    for qch in range(NQH):
                        qcs = [qc for qc in (2 * qch, 2 * qch + 1) if qc < NQC]
                        nki_max = min(4 * (qcs[-1] + 1), NKI)
                        for hl in range(HL):
                            qT_sb = qp.tile([128, len(qcs) * QC], f32r, tag="qT",
                                            name=f"qT_{b}_{qch}_{hl}")
                            nc.sync.dma_start(
                                qT_sb[:],
                                q_t.bitcast(f32r)[hl * 128:(hl + 1) * 128,
                                                  b * S + qcs[0] * QC: b * S + (qcs[-1] + 1) * QC])
                            kT_sb = qp.tile([128, nki_max * 128], f32r, tag="kT",
                                            name=f"kT_{b}_{qch}_{hl}")
                            nc.sync.dma_start(
                                kT_sb[:],
                                k_t.bitcast(f32r)[hl * 128:(hl + 1) * 128,
                                                  b * S: b * S + nki_max * 128])
                            v_sb = qp.tile([128, nki_max * 128], f32r, tag="v",
                                           name=f"v_{b}_{qch}_{hl}")
                            nc.sync.dma_start(
                                v_sb[:].rearrange("p (ki j) -> p ki j", ki=nki_max),
                                v_n_r[:, b * NKI: b * NKI + nki_max, hl * 128:(hl + 1) * 128],
                            )
                            for qc in qcs:
                                nki_q = min(4 * (qc + 1), NKI)
                                qoff = (qc - qcs[0]) * QC
                                probs = []
                                acc = accp.tile([128, QC], f32, tag="acc")
                                for ki in range(nki_q):
                                    p_sc = psc.tile([128, QC], f32, tag="sc")
                                    nc.tensor.matmul(
                                        p_sc[:],
                                        kT_sb[:, ki * 128:(ki + 1) * 128],
                                        qT_sb[:, qoff:qoff + QC],
                                        start=True, stop=True,
                                    )
                                    j = ki - (qc * QC) // 128
                                    if j >= 0:
                                        nc.vector.tensor_add(p_sc[:], p_sc[:], mb_sb[:, j * 512:(j + 1) * 512])
                                    pt_ = prp.tile([128, QC], f32r, tag="probs",
                                                   name=f"probs_{b}_{hl}_{qc}_{ki}")
                                    nc.scalar.activation(pt_[:], p_sc[:], Exp, scale=scale)
                                    if ki == 0:
                                        nc.vector.tensor_copy(acc[:], pt_[:].bitcast(f32))
                                    else:
                                        nc.vector.tensor_add(acc[:], acc[:], pt_[:].bitcast(f32))
                                    probs.append(pt_)
                                p_out = pso.tile([128, QC], f32, tag="po")
                                p_l = psl.tile([1, QC], f32, tag="pl")
                                for ki in range(nki_q):
                                    nc.tensor.matmul(
                                        p_out[:],
                                        v_sb[:, ki * 128:(ki + 1) * 128],
                                        probs[ki][:],
                                        start=(ki == 0), stop=(ki == nki_q - 1),
                                    )
                                nc.tensor.matmul(p_l[:], ones_col_f, acc[:], start=True, stop=True)
                                r_sb = op_.tile([1, QC], bf16, tag="r")
                                with nc.allow_low_precision(reason="1/l broadcast in bf16 is intentional"):
                                    nc.vector.reciprocal(r_sb[:], p_l[:])
                                p_r = psc.tile([128, QC], f32, tag="sc")
                                nc.tensor.matmul(p_r[:], onesb_sb[:], r_sb[:], start=True, stop=True)
                                rbc = op_.tile([128, QC], f32, tag="rbc")
                                nc.vector.tensor_copy(rbc[:], p_r[:])
                                attn = op_.tile([128, QC], bf16, tag="attn")
                                nc.vector.tensor_mul(attn[:], p_out[:], rbc[:])
                                tok0 = b * S + qc * QC
                                j = tok0 // AGC
                                col = tok0 % AGC
                                nc.sync.dma_start(
                                    ag_in[j][hl * 128:(hl + 1) * 128, col:col + QC], attn[:]
                                )
                                if hl == HL - 1:
                                    for jj in range(NAG):
                                        last_b = min(B - 1, (AGC * (jj + 1) - 1) // S)
                                        last_qc = min(NQC - 1, (min(AGC * (jj + 1), (last_b + 1) * S) - 1 - last_b * S) // QC)
                                        if last_b == b and last_qc == qc:
                                            nc.gpsimd.collective_compute(
                                                "AllGather",
                                                mybir.AluOpType.bypass,
                                                replica_groups=[list(range(NC))],
                                                ins=[ag_in[jj].opt()],
                                                outs=[ag_out[jj].opt()],
                                            )

                with tc.tile_pool(name="p2w", bufs=1) as wpv, \
                     tc.tile_pool(name="p2x", bufs=2) as xpv, \
                     tc.tile_pool(name="p2o", bufs=NM) as vop, \
                     tc.tile_pool(name="p2ps", bufs=max(NM, TOKC // 128), space="PSUM") as ppv:
                    wv_sb = load_w_slab(wpv, wT["v"], f32r, "wv")
                    emit_v_chunks(wv_sb, xpv, vop, ppv, range(0, NTC // B))
                    emit_attn_batch(0)
                    emit_v_chunks(wv_sb, xpv, vop, ppv, range(NTC // B, NTC))

                # ---------- P4 (pools take over the v-projection's space) ----
                with tc.tile_pool(name="p4w", bufs=1) as wp4, \
                     tc.tile_pool(name="p4x", bufs=5) as xp4, \
                     tc.tile_pool(name="p4o", bufs=2) as op4, \
                     tc.tile_pool(name="p4ps", bufs=min(NM, 4), space="PSUM") as pp4:
                    wo_sb = load_w_slab(wp4, woT, bf16, "wo")
                    for bb in range(1, B):
                        emit_attn_batch(bb)
                    ag_out_r = [a.rearrange("(dt p) t -> p dt t", p=128) for a in ag_out]
                    for tokc in range(NTC):
                        j = (tokc * TOKC) // AGC
                        col = (tokc * TOKC) % AGC
                        xg = []
                        for g in range(NG):
                            t = xp4.tile([128, XG * TOKC], bf16, tag="ag", name=f"agx_{tokc}_{g}")
                            nc.sync.dma_start(
                                t[:].rearrange("p (i t) -> p i t", i=XG),
                                ag_out_r[j][:, g * XG:(g + 1) * XG, col:col + TOKC],
                            )
                            xg.append(t)
                        psums = {m: pp4.tile([128, TOKC], f32, tag="pw", name=f"psw_{tokc}_{m}") for m in range(NM)}
                        for dt in range(ND):
                            g, i = divmod(dt, XG)
                            rhs = xg[g][:, i * TOKC:(i + 1) * TOKC]
                            for m in range(NM):
                                nc.tensor.matmul(
                                    psums[m][:],
                                    wo_sb[:, dt * DL + m * 128: dt * DL + (m + 1) * 128],
                                    rhs,
                                    start=(dt == 0), stop=(dt == ND - 1),
                                )
                        for m in range(NM):
                            ow = op4.tile([128, TOKC], f32, tag="ow")
                            nc.scalar.copy(ow[:], psums[m][:])
                            nc.sync.dma_start(
                                out_t[m * 128:(m + 1) * 128, tokc * TOKC:(tokc + 1) * TOKC],
                                ow[:],
                            )

    nc.compile()
    meta = dict(D=D, S=S, B=B, H=H, HL=HL, DL=DL, NB=NB)
    return nc, meta


def make_inputs(x, freqs_cis, wq, wk, wv, wo, D, S, B, H):
    """Host-side sharding: build the per-core in_maps."""
    import ml_dtypes

    NC = N_CORES
    TOK = B * S
    HL = H // NC
    DL = HL * 128
    NB = 4

    xT = np.ascontiguousarray(x.reshape(TOK, D).T)

    fc = np.asarray(freqs_cis, np.float32)           # (S, 64, 2)
    cmat = np.repeat(np.ascontiguousarray(fc[:, :, 0].T), 2, axis=0).astype(np.float32)
    smat = np.repeat(np.ascontiguousarray(fc[:, :, 1].T), 2, axis=0).astype(np.float32)

    P = np.zeros((128, 128), np.float32)
    idx = np.arange(64)
    P[2 * idx, 2 * idx + 1] = -1.0
    P[2 * idx + 1, 2 * idx] = 1.0
    consts = np.zeros((128, 512), np.float32)
    consts[:, 0:128] = P.T
    consts[:, 128] = 1.0                     # ones_col (f32r view)
    consts[:, 129:257] = np.eye(128, dtype=np.float32)
    consts[0, 257:385] = 1.0                 # ones_row
    consts[:, 385] = 1.0                     # ones_col (f32)

    maskb = np.zeros((128, NB * 512), np.float32)
    p = np.arange(128)[:, None]
    f = np.arange(512)[None, :]
    for j in range(NB):
        maskb[:, j * 512:(j + 1) * 512] = np.where(128 * j + p <= f, 0.0, -1e9)

    in_maps = []
    for c in range(NC):
        sl = slice(c * DL, (c + 1) * DL)
        in_maps.append({
            "xT": xT,
            "wqT": np.ascontiguousarray(wq[sl, :].T),
            "wkT": np.ascontiguousarray(wk[sl, :].T),
            "wvT": np.ascontiguousarray(wv[sl, :].T),
            "woT": np.ascontiguousarray(wo[sl, :].T).astype(ml_dtypes.bfloat16),
            "cmat": cmat,
            "smat": smat,
            "consts": consts,
            "onesb": np.ones((1, 128), ml_dtypes.bfloat16),
            "maskb": maskb,
        })
    return in_maps


def assemble_outputs(results, D, S, B, H):
    """Gather per-core outputs into (out, new_hidden)."""
    NC = N_CORES
    TOK = B * S
    HL = H // NC
    DL = HL * 128

    out = np.empty((TOK, D), np.float32)
    k_full = np.empty((B, S, H, 128), np.float32)
    v_full = np.empty((B, S, H, 128), np.float32)
    for c in range(NC):
        r = results[c]
        out[:, c * DL:(c + 1) * DL] = np.asarray(r["out_t"], np.float32).T
        kt = np.asarray(r["k_t"], np.float32).reshape(HL, 128, B, S).transpose(2, 3, 0, 1)
        k_full[:, :, c * HL:(c + 1) * HL, :] = kt
        v_full[:, :, c * HL:(c + 1) * HL, :] = np.asarray(r["v_n"], np.float32).reshape(B, S, HL, 128)
    out = out.reshape(B, S, D)
    new_hidden = np.stack([k_full, v_full], axis=0)
    return out, new_hidden


def _reference_fallback(x, freqs_cis, mask, hidden_state, wq, wk, wv, wo, start_pos):
    """Pure-numpy mirror of the reference for inputs the fast path can't take."""
    bsz, seqlen, dim = x.shape
    h, hd = dim // 128, 128
    xt = x.reshape(bsz * seqlen, dim).astype(np.float32)
    q = (xt @ wq.T).reshape(bsz, seqlen, h, hd)
    k = (xt @ wk.T).reshape(bsz, seqlen, h, hd)
    v = (xt @ wv.T).reshape(bsz, seqlen, h, hd)
    fc = np.asarray(freqs_cis, np.float32)

    def rot(z):
        zz = z.reshape(bsz, seqlen, h, hd // 2, 2)
        cos = fc[:, :, 0][None, :, None, :]
        sin = fc[:, :, 1][None, :, None, :]
        r = zz[..., 0] * cos - zz[..., 1] * sin
        im = zz[..., 0] * sin + zz[..., 1] * cos
        return np.stack([r, im], -1).reshape(bsz, seqlen, h, hd).astype(np.float32)

    qr, kr = rot(q), rot(k)
    new_k = np.array(hidden_state[0])
    new_v = np.array(hidden_state[1])
    new_k[:bsz, start_pos:start_pos + seqlen] = kr
    new_v[:bsz, start_pos:start_pos + seqlen] = v
    keys = new_k[:bsz, :start_pos + seqlen]
    values = new_v[:bsz, :start_pos + seqlen]
    out = np.zeros((bsz, seqlen, h, hd), np.float32)
    m2 = np.asarray(mask, np.float32)[0, 0]
    for b in range(bsz):
        for hh in range(h):
            sc = (qr[b, :, hh, :] @ keys[b, :, hh, :].T) / math.sqrt(hd) + m2
            sc = sc - sc.max(-1, keepdims=True)
            p = np.exp(sc)
            p = p / p.sum(-1, keepdims=True)
            out[b, :, hh, :] = p @ values[b, :, hh, :]
    return (out.reshape(bsz, seqlen, h * hd) @ wo.T,
            np.stack([new_k, new_v], 0))


def _is_causal_mask(mask, S):
    m = np.asarray(mask, np.float32).reshape(S, S)
    expect = np.triu(np.full((S, S), -1e9, np.float32), k=1)
    return m.shape == expect.shape and np.array_equal(m, expect)


def run_on_device(in_maps, prog_key=None, trace=False):
    from concourse.bass_utils import run_bass_kernel_spmd

    key = prog_key or tuple(sorted(FULL.items()))
    if key not in _PROG_CACHE:
        _PROG_CACHE[key] = build_program(**dict(key))
    nc, meta = _PROG_CACHE[key]
    res = run_bass_kernel_spmd(
        nc, in_maps, core_ids=list(range(N_CORES)), trace=trace
    )
    return res, meta


def kernel(x, freqs_cis, mask, hidden_state, wq, wk, wv, wo, start_pos):
    x = np.asarray(x, np.float32)
    freqs_cis = np.asarray(freqs_cis, np.float32)
    wq = np.asarray(wq, np.float32)
    wk = np.asarray(wk, np.float32)
    wv = np.asarray(wv, np.float32)
    wo = np.asarray(wo, np.float32)
    D, S, B, H = FULL["D"], FULL["S"], FULL["B"], FULL["H"]

    sp = int(start_pos)
    if (sp != 0 or x.shape != (B, S, D) or not _is_causal_mask(mask, S)):
        return _reference_fallback(
            x, freqs_cis, np.asarray(mask), np.asarray(hidden_state),
            wq, wk, wv, wo, sp)

    in_maps = make_inputs(x, freqs_cis, wq, wk, wv, wo, D, S, B, H)
    res, _meta = run_on_device(in_maps)
    out, new_hidden = assemble_outputs(res.results, D, S, B, H)
    return out, new_hidden
